# revision 1
# baseline (speedup 1.0000x reference)
"""Neural CDE forward pass on 8 Trainium2 NeuronCores (Bass/Tile).

Math (per batch element b):
    z0 = u0 @ Wi + bi                                   [64]
    for t in 0..164:
        h  = relu(z @ W1 + b1)                          [128]
        f  = tanh(h @ W2 + b2)                          [512] -> [64, 8]
        z += einsum('hi,i->h', f, dx_t)                 dx_t = coeffs[t+1]-coeffs[t]
    out_t = z_t @ Wr + br  for every t (166 values)

Numerics/perf model (all hardware-measured on this container):
  - The scan is chaotic: errors amplify ~1.05x/step (~3000x over 165 steps).
  - fp32 matmul: exact-grade (1e-7 rel/step) but 4 cycles/row: 559 ns per
    N=256 matmul including the serialized fused weight load.
  - float32r matmul: operands rounded to ~12 mantissa bits (1.4e-4
    rel/step) but 1 cycle/row: 223 ns per N=256 matmul.  An all-f32r scan
    measures 21.3 absmax final error (vs 1.25 allowed = 2e-2 * 62.5).
  - Hybrid phase split: step-t errors are amplified by ~1.05^(165-t), so
    running fp32 for t < T0 and f32r for t >= T0 gives final error
    ~ 5.5e-2 (fp32 part) + 21.3 * 1.05^-T0.  measured absmax vs T0:
    0.321@72, 0.491@65, 0.627@59 (shipped: rel 1.0e-2, 2x inside the
    gate), 1.363@56 - the growth steepens sharply below T0~58, so 59 is
    the knee.  A numpy simulation of the rounding semantics reproduces
    the measured error and shows (a) the late-phase error is spread
    evenly across the h/W2/g roundings - no selective extra pass pays
    for itself - and (b) ranking all 165 steps by per-step cost
    (injected rounding error x remaining amplification, both simulated)
    selects exactly the contiguous prefix {0..58} as the optimal fp32
    set, and shrinking it to 50 steps triples the error: the contiguous
    T0=59 split is the optimum, not an approximation.  Rounding the
    late-phase mm1 input (f32r z via a twin add) measures x1.87 error -
    rel 1.96e-2, too close to the gate - so mm1 stays fp32 in both
    phases: the state stream is the most amplification-sensitive input.
    Single-step f32r flips inside the fp32 prefix were also probed in sim
    (deterministic cancellation exists - flipping step 55 improves the
    error) but at most ~2 steps (~7 us) are exploitable: below noise and
    fragile to sim-vs-HW rounding-phase differences, so not taken.
    Measured end-to-end: ~1.0-1.1 ms vs the 1.78 ms fp32 baseline (513-
    scan hardware-loop wall, the cleanest comparable, dropped 2.14 s ->
    1.95 s across the final optimization sequence).  The f32r phase is
    bound by the z->h->f->g->e->z dependency cycle - dominated by the
    ScalarE queue (relu + 4 tanh) feeding the g multiplies - so the late
    phase writes tanh output in fp16 (halves the ACT write bytes; f only
    feeds the g multiply, whose result is f32r-rounded to 12 bits anyway,
    so the precision cost is ~5%).  The fp32 phase is PE-capacity-bound.

Kernel design (per core, batch shard B=512 split into NCHAIN=2 chains of
Bc=256 on the matmul free dim):
  - State zT [64+1, Bc] fp32 in SBUF per chain; row 64 carries the running
    readout out_t = z_t @ Wr + br.  mm1 (z -> h pre-act) is always fp32 so
    the state stream never loses precision.
  - h: ScalarE relu with fused per-partition bias b1 -> h tile (fp32 in
    the fp32 phase; declared f32r in the f32r phase - the PE rounds f32r
    operands internally, so no separate rounding op is needed).
  - f: 4 banks of W2.T h (single matmul per bank; fp32 or f32r stationary
    w2_sb / w2_13), tanh with bias b2_j on ScalarE -> f (fp32 early,
    fp16 late).
  - einsum: g_j = f_j * dx_rep on VectorE (fp32 or f32r out);
    dx_rep[p, b] = dx[b, p % 8] is DMA'd per step as [8, B] from HBM and
    partition-replicated 16x by the DMA itself (stride-0 source AP).
    e = sum_j S_j'.T g_j: 4 accumulating matmuls; S_j' [128, 65] has 0/1
    entries (exact in f32r) plus column 64 = S_j @ Wr whose rounding only
    touches the readout, so e[64] = Wr . e_z rides along for free.
  - z_new = z_old + e (VectorE fp32 add); row 64 is DMA'd per step.
  - Pipeline / tiling: the two chains share every stationary operand and
    every per-bank bias, so all wide ops fuse across chains: the two relus
    write halves of ONE [128, 2, Bc] h tile; mm2 is ONE N=512 matmul per
    bank; tanh and the g multiply are ONE [128, 512] op per bank; the
    reduce is ONE N=512 matmul per bank into a shared [65, 2, Bc] e_ps
    whose halves feed the per-chain z adds.  10 PE instructions and 12
    elementwise instructions per step (vs 18+18 in the per-chain form),
    which matters because every matmul carries ~130 ns of fixed
    weight-load/dispatch cost and every engine op ~100-200 ns.  Only mm1,
    relu and the z add stay per-chain: they gate the recurrence cycle, and
    splitting them keeps chain c1's state update off chain c0's critical
    path.  Values are bitwise identical to the per-chain emission.
"""

import numpy as np

IN_CH = 8
HID = 64
MLP_W = 128
OUT = 1
B_FULL, T = 4096, 166
NSTEP = T - 1
N_CORES = 8
B = B_FULL // N_CORES  # 512
NBANK = 4

# engine assignment knobs ("vector" | "gpsimd")
ENG_GMUL = ("vector", "gpsimd", "vector", "gpsimd")
ENG_G13 = ("gpsimd", "vector", "gpsimd", "vector")
ENG_DG = ("vector", "gpsimd", "vector", "gpsimd")
ADD_ON = "vector"
T0 = 59  # steps < T0 run fp32 matmuls; steps >= T0 run f32r
NCHAIN = 2
Bc = B // NCHAIN  # 256
DX_PREFETCH = 6
REPEAT = 1

_CACHE = {}


def _build_bass(repeat=1, knobs=None):
    from contextlib import ExitStack

    import concourse.tile as tile
    from concourse import bacc, mybir

    kn = dict(gmul=ENG_GMUL, g13=ENG_G13, dg=ENG_DG, add=ADD_ON, t0=T0)
    if knobs:
        kn.update(knobs)

    f32 = mybir.dt.float32
    f32r = mybir.dt.float32r
    f16 = mybir.dt.float16
    AF = mybir.ActivationFunctionType
    ALU = mybir.AluOpType

    nc = bacc.Bacc("TRN2", target_bir_lowering=False, debug=False)

    u0t = nc.dram_tensor("u0t", [IN_CH, B], f32, kind="ExternalInput")
    dxt = nc.dram_tensor("dxt", [NSTEP, IN_CH, B], f32, kind="ExternalInput")
    w1 = nc.dram_tensor("w1", [HID, MLP_W], f32, kind="ExternalInput")
    b1 = nc.dram_tensor("b1", [MLP_W, 1], f32, kind="ExternalInput")
    w2 = nc.dram_tensor("w2", [MLP_W, NBANK, 128], f32, kind="ExternalInput")
    b2 = nc.dram_tensor("b2", [128, NBANK], f32, kind="ExternalInput")
    wi = nc.dram_tensor("wi", [IN_CH, HID + 1], f32, kind="ExternalInput")
    smat = nc.dram_tensor("smat", [128, NBANK, HID + 1], f32,
                          kind="ExternalInput")
    outp = nc.dram_tensor("outp", [T, B], f32, kind="ExternalOutput")

    with tile.TileContext(nc) as tc, ExitStack() as ctx:
        const = ctx.enter_context(tc.tile_pool(name="const", bufs=1))
        zpool = ctx.enter_context(tc.tile_pool(name="zpool", bufs=2))
        hpool = ctx.enter_context(tc.tile_pool(name="hpool", bufs=2))
        fpool = ctx.enter_context(tc.tile_pool(name="fpool", bufs=2))
        gpool = ctx.enter_context(tc.tile_pool(name="gpool", bufs=3))
        dxpool = ctx.enter_context(tc.tile_pool(name="dxpool", bufs=8))
        psum_h = ctx.enter_context(tc.tile_pool(name="psum_h", bufs=2, space="PSUM"))
        psum_f = ctx.enter_context(tc.tile_pool(name="psum_f", bufs=2, space="PSUM"))
        psum_e = ctx.enter_context(tc.tile_pool(name="psum_e", bufs=2, space="PSUM"))

        w1_sb = const.tile([HID, MLP_W], f32)
        nc.sync.dma_start(w1_sb[:], w1[:])
        b1_sb = const.tile([MLP_W, 1], f32)
        nc.sync.dma_start(b1_sb[:], b1[:])
        w2_sb = const.tile([MLP_W, NBANK, 128], f32)
        nc.sync.dma_start(w2_sb[:], w2[:])
        b2_sb = const.tile([128, NBANK], f32)
        nc.sync.dma_start(b2_sb[:], b2[:])
        wi_sb = const.tile([IN_CH, HID + 1], f32)
        nc.sync.dma_start(wi_sb[:], wi[:])
        s_sb = const.tile([128, NBANK, HID + 1], f32)
        nc.sync.dma_start(s_sb[:], smat[:])
        s_sb_r = const.tile([128, NBANK, HID + 1], f32r, name="s_sb_r")
        nc.vector.tensor_copy(s_sb_r[:], s_sb[:])
        u0t_sb = const.tile([IN_CH, B], f32)
        nc.sync.dma_start(u0t_sb[:], u0t[:])

        # f32r alias of W2 for the late phase (PE rounds internally; the
        # DVE copy applies the same rounding, value-identical)
        w2_13 = const.tile([MLP_W, NBANK, 128], f32r, name="w2_13")
        nc.vector.tensor_copy(w2_13[:], w2_sb[:])

        z_sb = [None] * NCHAIN
        dx_tiles = {}
        g_banks = [None] * NBANK

        def init_chains():
            z0_ps = psum_e.tile([HID + 1, NCHAIN, Bc], f32, tag="e_ps",
                                name="z0_ps")
            for c in range(NCHAIN):
                cs = slice(c * Bc, (c + 1) * Bc)
                nc.tensor.matmul(
                    z0_ps[:, c, :], wi_sb[:], u0t_sb[:, cs],
                    start=True, stop=True
                )
                z_c = zpool.tile([HID + 1, Bc], f32, tag=f"z{c}", name=f"z_sb{c}")
                nc.vector.tensor_copy(z_c[:], z0_ps[:, c, :])
                nc.sync.dma_start(outp[0:1, cs], z_c[HID : HID + 1, :])
                z_sb[c] = z_c

        def frag_mm1_h(c, t, lo, h_tile):
            """fp32 mm1 per chain; relu+bias (DVE) writes this chain's half
            of the shared h tile."""
            h_ps = psum_h.tile([MLP_W, Bc], f32, tag="h_ps", name="h_ps")
            nc.tensor.matmul(
                h_ps[:], w1_sb[:], z_sb[c][0:HID, :], start=True, stop=True
            )
            # ScalarE: closer to PSUM, and its queue is idle here (the DVE
            # queue still holds the z adds that gate this step's mm1s)
            nc.scalar.activation(
                h_tile[:, c, :], h_ps[:], AF.Relu, bias=b1_sb[:, 0:1]
            )

        def frag_mm2_g(t, h_tile, lo):
            """per bank: both chains' matmuls into one PSUM tile, then ONE
            [128, 512] tanh (b2_j is per-bank, same for both chains) and
            ONE [128, 512] g multiply against the full dx tile."""
            dx_sb = dx_tiles[t]
            w2_use = w2_13 if lo else w2_sb
            for j in range(NBANK):
                f_ps = psum_f.tile([128, NCHAIN, Bc], f32, tag=f"f_ps{j}",
                                   bufs=1, name=f"f_ps{j}")
                nc.tensor.matmul(f_ps[:], w2_use[:, j, :], h_tile[:],
                                 start=True, stop=True)
                f_sb = fpool.tile([128, NCHAIN, Bc], f16 if lo else f32,
                                  tag=f"f_r{j}" if lo else f"f_f{j}",
                                  name=f"f_sb{j}")
                nc.scalar.activation(
                    f_sb[:], f_ps[:], AF.Tanh, bias=b2_sb[:, j : j + 1]
                )
                g_sb = gpool.tile([128, NCHAIN, Bc], f32r if lo else f32,
                                  tag=f"g_r{j}" if lo else f"g_f{j}",
                                  name=f"g_sb{j}")
                getattr(nc, kn["gmul"][j]).tensor_mul(g_sb[:], f_sb[:], dx_sb[:])
                g_banks[j] = g_sb

        def frag_red_both(t, lo):
            e_ps = psum_e.tile([HID + 1, NCHAIN, Bc], f32, tag="e_ps",
                               name="e_ps")
            s_use = s_sb_r if lo else s_sb
            for j in range(NBANK):
                nc.tensor.matmul(e_ps[:], s_use[:, j, :], g_banks[j][:],
                                 start=j == 0, stop=j == NBANK - 1)
            for c in range(NCHAIN):
                cs = slice(c * Bc, (c + 1) * Bc)
                z_prev = z_sb[c]
                z_sb[c] = zpool.tile([HID + 1, Bc], f32, tag=f"z{c}",
                                     name=f"z_sb{c}")
                getattr(nc, kn["add"]).tensor_add(
                    z_sb[c][:], e_ps[:, c, :], z_prev[:]
                )
                nc.sync.dma_start(outp[t + 1 : t + 2, cs],
                                  z_sb[c][HID : HID + 1, :])

        def dma_dx(t):
            if t >= NSTEP:
                return
            dx_sb = dxpool.tile([128, B], f32, tag="dx", name="dx_sb")
            nc.sync.dma_start(
                dx_sb[:], dxt[t][None, :, :].to_broadcast([128 // IN_CH, IN_CH, B])
            )
            dx_tiles[t] = dx_sb
            if t - DX_PREFETCH - 1 in dx_tiles:
                del dx_tiles[t - DX_PREFETCH - 1]

        def scan_body():
            init_chains()
            dx_tiles.clear()
            for t in range(DX_PREFETCH):
                dma_dx(t)
            for t in range(NSTEP):
                lo = t >= kn["t0"]
                dma_dx(t + DX_PREFETCH)
                h_tile = hpool.tile([MLP_W, NCHAIN, Bc], f32r if lo else f32,
                                    tag="h_r" if lo else "h_f", name="h_tile")
                frag_mm1_h(0, t, lo, h_tile)
                frag_mm1_h(1, t, lo, h_tile)
                frag_mm2_g(t, h_tile, lo)
                frag_red_both(t, lo)

        if repeat == 1:
            scan_body()
        else:
            # hardware loop: trip count is a runtime scalar, so timing
            # amplification costs no extra instructions
            with tc.For_i(0, repeat):
                scan_body()

    nc.compile()
    return nc


def _prep_host(u0, coeffs, W1, b1, W2, b2, Wi, bi, Wr, br):
    f32 = np.float32

    u0t_full = np.empty((IN_CH, B_FULL), f32)
    u0t_full[: IN_CH - 1] = u0.T
    u0t_full[IN_CH - 1] = 1.0

    dX = (coeffs[:, 1:] - coeffs[:, :-1]).astype(f32)  # [B_FULL, NSTEP, IN_CH]
    dxt_small = np.ascontiguousarray(dX.transpose(1, 2, 0))  # [NSTEP, 8, B_FULL]

    wi_mat = np.empty((IN_CH, HID + 1), f32)
    wi_mat[: IN_CH - 1, :HID] = Wi
    wi_mat[IN_CH - 1, :HID] = bi
    wi_mat[: IN_CH - 1, HID] = (Wi @ Wr)[:, 0]
    wi_mat[IN_CH - 1, HID] = float(bi @ Wr[:, 0] + br[0])

    w2_banks = np.ascontiguousarray(W2.reshape(MLP_W, NBANK, 128))
    b2_banks = np.ascontiguousarray(b2.reshape(NBANK, 128).T)

    p = np.arange(128)
    s_full = np.zeros((128, NBANK, HID + 1), f32)
    for j in range(NBANK):
        s_full[p, j, 16 * j + p // IN_CH] = 1.0
        s_full[p, j, HID] = Wr[16 * j + p // IN_CH, 0]

    return {
        "u0t": u0t_full,
        "dxt": dxt_small,
        "w1": np.ascontiguousarray(W1.astype(f32)),
        "b1": np.ascontiguousarray(b1.astype(f32).reshape(MLP_W, 1)),
        "w2": w2_banks.astype(f32),
        "b2": b2_banks.astype(f32),
        "wi": wi_mat,
        "smat": s_full,
    }


def _make_in_maps(full):
    in_maps = []
    for c in range(N_CORES):
        sl = slice(c * B, (c + 1) * B)
        in_maps.append(
            {
                "u0t": np.ascontiguousarray(full["u0t"][:, sl]),
                "dxt": np.ascontiguousarray(full["dxt"][:, :, sl]),
                "w1": full["w1"],
                "b1": full["b1"],
                "w2": full["w2"],
                "b2": full["b2"],
                "wi": full["wi"],
                "smat": full["smat"],
            }
        )
    return in_maps


def kernel(u0, coeffs, W1, b1, W2, b2, Wi, bi, Wr, br, repeat=None, knobs=None):
    from concourse.bass_utils import run_bass_kernel_spmd

    full = _prep_host(
        np.asarray(u0, np.float32), np.asarray(coeffs, np.float32),
        np.asarray(W1, np.float32), np.asarray(b1, np.float32),
        np.asarray(W2, np.float32), np.asarray(b2, np.float32),
        np.asarray(Wi, np.float32), np.asarray(bi, np.float32),
        np.asarray(Wr, np.float32).reshape(HID, OUT),
        np.asarray(br, np.float32).reshape(OUT),
    )
    in_maps = _make_in_maps(full)

    rep = REPEAT if repeat is None else repeat
    key = ("nc", rep, tuple(sorted(knobs.items())) if knobs else None)
    if key not in _CACHE:
        _CACHE[key] = _build_bass(rep, knobs)
    nc = _CACHE[key]

    res = run_bass_kernel_spmd(nc, in_maps, core_ids=list(range(N_CORES)))
    outs = res.results

    out_full = np.empty((B_FULL, T, OUT), np.float32)
    for c in range(N_CORES):
        out_full[c * B : (c + 1) * B, :, 0] = outs[c]["outp"].T
    return out_full



# revision 11
# speedup vs baseline: 1.1298x; 1.1298x over previous
"""Neural CDE forward pass on 8 Trainium2 NeuronCores (Bass/Tile).

Math (per batch element b):
    z0 = u0 @ Wi + bi                                   [64]
    for t in 0..164:
        h  = relu(z @ W1 + b1)                          [128]
        f  = tanh(h @ W2 + b2)                          [512] -> [64, 8]
        z += einsum('hi,i->h', f, dx_t)                 dx_t = coeffs[t+1]-coeffs[t]
    out_t = z_t @ Wr + br  for every t (166 values)

Numerics (hardware-measured, see git history of this docstring):
  - The scan is chaotic: errors amplify ~1.05x/step (~3000x over 165 steps).
  - fp32 matmul: exact-grade but 4 cycles/row; float32r: operands rounded
    to ~12 mantissa bits (1.4e-4 rel/step) at 1 cycle/row.
  - Hybrid phase split: steps t < T0=59 run fp32 mm2/reduce, t >= T0 run
    f32r.  Final error ~1e-2 rel (gate 2e-2).  T0=59 is the measured knee;
    mm1 (the z state stream) stays fp32 in both phases.

Kernel design (per core, batch shard B=512 in NCHAIN=2 chains of Bc=256):
  - State z [66, Bc] fp32: rows 0..63 state, row 64 == 1.0 (carries b1
    into mm1: stationary w1b[65,128] = [W1; b1], moving z[0:65]), row 65 =
    running readout out_t = z_t @ Wr + br.  The 1.0 row means relu needs
    NO fused bias, so it can run on any engine.  The reduce matrices S_j
    [128, 66] get a zero column at 64 (keeps the 1.0 row fixed) and the
    Wr column at 65 (readout rides the reduce for free).
  - States are slotted: state s lives at z_st[c][:, s % NSLOT, :].  Output
    row 65 drains with ONE DMA per OUTB=4 states per chain, and dx loads
    arrive DXB=4 steps per DMA (HWDGE fixed cost ~625ns/DMA; batching
    keeps the SP queue + HWDGE device off the critical path).
  - fp32 phase (t < T0): PE-bound; both chains fused into wide [*, 512]
    ops (one relu half per chain, one mm2/tanh/gmul/reduce per bank) to
    minimize PE instruction count.
  - f32r phase (t >= T0): latency-bound on the z->h->f->g->e->z cycle.
    Emission switches to per-chain ops (knob lsplit): each chain becomes
    an independent recurrence pipeline (mm2/tanh/gmul/reduce per chain
    per bank at [*, 256]); the two pipelines interleave on the engines,
    hiding each other's serial latency.  PSUM tiles keep the fused
    [.., NCHAIN, Bc] shape with per-chain half-views so both phases share
    the same PSUM budget (14KB of 16KB).
  - Engine knobs spread elementwise work: relu/gmul/z-add per chain on
    vector vs gpsimd vs scalar (tuned via the cost-model TimelineSim,
    verified on hardware).
"""

import numpy as np

IN_CH = 8
HID = 64
MLP_W = 128
OUT = 1
B_FULL, T = 4096, 166
NSTEP = T - 1
N_CORES = 8
B = B_FULL // N_CORES  # 512
NBANK = 4
HID1 = HID + 2  # 64 state rows + const-one row (64) + readout row (65)
ROW_ONE = HID
ROW_OUT = HID + 1

T0 = 59  # steps < T0 run fp32 matmuls; steps >= T0 run f32r
NCHAIN = 2
Bc = B // NCHAIN  # 256
DXB = 4  # scan steps per dx DMA block (batched to amortize HWDGE overhead)
NDXBLK = (NSTEP + DXB - 1) // DXB  # 42 (last block zero-padded)
DXBLK_PREFETCH = 2
NSLOT = 8  # z state slots per chain (state s lives at slot s % NSLOT)
OUTB = 4  # states per output DMA (4 divides NSLOT so slot runs stay contiguous)
REPEAT = 1

# engine assignment knobs ("vector" | "gpsimd" | "scalar" where noted)
KNOBS = dict(
    t0=T0,
    lsplit=True,                      # per-chain emission in the f32r phase
    relu_e="scalar",                  # fp32-phase relu engine
    # PSUM readers (relu, z-add) are restricted to vector/scalar — GPSIMD
    # has no PSUM port.  gmul reads SBUF only, so it may use gpsimd.
    relu_l=("scalar", "vector"),      # f32r-phase relu engine per chain
    gmul_e=("vector", "gpsimd", "vector", "gpsimd"),  # fp32 phase, per bank
    gmul_l=(("vector", "gpsimd", "vector", "gpsimd"),
            ("gpsimd", "vector", "gpsimd", "vector")),  # f32r, [chain][bank]
    add_e=("vector", "vector"),       # per chain, both phases
)

_CACHE = {}


def _build_bass(repeat=1, knobs=None):
    from contextlib import ExitStack

    import concourse.tile as tile
    from concourse import bacc, mybir

    kn = dict(KNOBS)
    if knobs:
        kn.update(knobs)

    f32 = mybir.dt.float32
    f32r = mybir.dt.float32r
    f16 = mybir.dt.float16
    AF = mybir.ActivationFunctionType

    nc = bacc.Bacc("TRN2", target_bir_lowering=False, debug=False)

    u0t = nc.dram_tensor("u0t", [IN_CH, B], f32, kind="ExternalInput")
    dxt = nc.dram_tensor("dxt", [NDXBLK, IN_CH, DXB, B], f32,
                         kind="ExternalInput")
    w1b = nc.dram_tensor("w1b", [HID + 1, MLP_W], f32, kind="ExternalInput")
    w2 = nc.dram_tensor("w2", [MLP_W, NBANK, 128], f32, kind="ExternalInput")
    b2 = nc.dram_tensor("b2", [128, NBANK], f32, kind="ExternalInput")
    wi = nc.dram_tensor("wi", [IN_CH, HID1], f32, kind="ExternalInput")
    smat = nc.dram_tensor("smat", [128, NBANK, HID1], f32,
                          kind="ExternalInput")
    outp = nc.dram_tensor("outp", [T, B], f32, kind="ExternalOutput")

    def eng(name):
        return getattr(nc, name)

    def relu_op(engine, dst, src):
        if engine == "scalar":
            nc.scalar.activation(dst, src, AF.Relu)
        else:
            eng(engine).tensor_relu(dst, src)

    with tile.TileContext(nc) as tc, ExitStack() as ctx:
        const = ctx.enter_context(tc.tile_pool(name="const", bufs=1))
        hpool = ctx.enter_context(tc.tile_pool(name="hpool", bufs=2))
        fpool = ctx.enter_context(tc.tile_pool(name="fpool", bufs=2))
        gpool = ctx.enter_context(tc.tile_pool(name="gpool", bufs=3))
        dxpool = ctx.enter_context(tc.tile_pool(name="dxpool", bufs=3))
        psum_h = ctx.enter_context(tc.tile_pool(name="psum_h", bufs=2, space="PSUM"))
        psum_f = ctx.enter_context(tc.tile_pool(name="psum_f", bufs=2, space="PSUM"))
        psum_e = ctx.enter_context(tc.tile_pool(name="psum_e", bufs=2, space="PSUM"))

        w1b_sb = const.tile([HID + 1, MLP_W], f32)
        nc.sync.dma_start(w1b_sb[:], w1b[:])
        w2_sb = const.tile([MLP_W, NBANK, 128], f32)
        nc.sync.dma_start(w2_sb[:], w2[:])
        b2_sb = const.tile([128, NBANK], f32)
        nc.sync.dma_start(b2_sb[:], b2[:])
        wi_sb = const.tile([IN_CH, HID1], f32)
        nc.sync.dma_start(wi_sb[:], wi[:])
        s_sb = const.tile([128, NBANK, HID1], f32)
        nc.sync.dma_start(s_sb[:], smat[:])
        s_sb_r = const.tile([128, NBANK, HID1], f32r, name="s_sb_r")
        nc.vector.tensor_copy(s_sb_r[:], s_sb[:])
        u0t_sb = const.tile([IN_CH, B], f32)
        nc.sync.dma_start(u0t_sb[:], u0t[:])

        # f32r alias of W2 for the late phase (PE rounds internally; the
        # DVE copy applies the same rounding, value-identical)
        w2_13 = const.tile([MLP_W, NBANK, 128], f32r, name="w2_13")
        nc.vector.tensor_copy(w2_13[:], w2_sb[:])

        # Slotted state: state s lives at z_st[c][:, s % NSLOT, :]
        z_st = [
            const.tile([HID1, NSLOT, Bc], f32, name=f"z_st{c}")
            for c in range(NCHAIN)
        ]
        dx_blks = {}
        g_banks = [None] * NBANK

        def init_chains():
            z0_ps = psum_e.tile([HID1, NCHAIN, Bc], f32, tag="e_ps",
                                name="z0_ps")
            for c in range(NCHAIN):
                cs = slice(c * Bc, (c + 1) * Bc)
                nc.tensor.matmul(
                    z0_ps[:, c, :], wi_sb[:], u0t_sb[:, cs],
                    start=True, stop=True
                )
                nc.vector.tensor_copy(z_st[c][:, 0, :], z0_ps[:, c, :])

        def drain_out(t):
            """one DMA per chain per OUTB states once state t+1 closes a
            group (or is the last state)"""
            s_hi = t + 1
            if s_hi % OUTB == OUTB - 1 or s_hi == NSTEP:
                s_lo = (s_hi // OUTB) * OUTB
                sl = s_lo % NSLOT
                n = s_hi - s_lo + 1
                for c in range(NCHAIN):
                    cs = slice(c * Bc, (c + 1) * Bc)
                    nc.sync.dma_start(
                        outp[s_lo : s_hi + 1, cs],
                        z_st[c][ROW_OUT : ROW_OUT + 1, sl : sl + n, :],
                    )

        def step_fused(t, lo):
            """both chains fused into wide ops (fp32 phase: PE-bound)"""
            h_tile = hpool.tile([MLP_W, NCHAIN, Bc], f32r if lo else f32,
                                tag="h_r" if lo else "h_f", name="h_tile")
            for c in range(NCHAIN):
                h_ps = psum_h.tile([MLP_W, Bc], f32, tag="h_ps", name="h_ps")
                nc.tensor.matmul(
                    h_ps[:], w1b_sb[:], z_st[c][0 : HID + 1, t % NSLOT, :],
                    start=True, stop=True
                )
                relu_op(kn["relu_e"], h_tile[:, c, :], h_ps[:])
            dx_sb = dx_blks[t // DXB][:, t % DXB, :]
            w2_use = w2_13 if lo else w2_sb
            for j in range(NBANK):
                f_ps = psum_f.tile([128, NCHAIN, Bc], f32, tag=f"f_ps{j}",
                                   bufs=1, name=f"f_ps{j}")
                nc.tensor.matmul(f_ps[:], w2_use[:, j, :], h_tile[:],
                                 start=True, stop=True)
                f_sb = fpool.tile([128, NCHAIN, Bc], f16 if lo else f32,
                                  tag=f"f_r{j}" if lo else f"f_f{j}",
                                  name=f"f_sb{j}")
                nc.scalar.activation(
                    f_sb[:], f_ps[:], AF.Tanh, bias=b2_sb[:, j : j + 1]
                )
                g_sb = gpool.tile([128, NCHAIN, Bc], f32r if lo else f32,
                                  tag=f"g_r{j}" if lo else f"g_f{j}",
                                  name=f"g_sb{j}")
                eng(kn["gmul_e"][j]).tensor_mul(g_sb[:], f_sb[:], dx_sb[:])
                g_banks[j] = g_sb
            e_ps = psum_e.tile([HID1, NCHAIN, Bc], f32, tag="e_ps",
                               name="e_ps")
            s_use = s_sb_r if lo else s_sb
            for j in range(NBANK):
                nc.tensor.matmul(e_ps[:], s_use[:, j, :], g_banks[j][:],
                                 start=j == 0, stop=j == NBANK - 1)
            for c in range(NCHAIN):
                eng(kn["add_e"][c]).tensor_add(
                    z_st[c][:, (t + 1) % NSLOT, :], e_ps[:, c, :],
                    z_st[c][:, t % NSLOT, :]
                )
            drain_out(t)

        def step_split(t, lo):
            """per-chain emission (f32r phase: latency-bound; the chains
            form two independent pipelines).  PSUM tiles keep the fused
            shape; each chain uses its half-view."""
            h_tile = hpool.tile([MLP_W, NCHAIN, Bc], f32r if lo else f32,
                                tag="h_r" if lo else "h_f", name="h_tile")
            f_ps = [
                psum_f.tile([128, NCHAIN, Bc], f32, tag=f"f_ps{j}",
                            bufs=1, name=f"f_ps{j}")
                for j in range(NBANK)
            ]
            e_ps = psum_e.tile([HID1, NCHAIN, Bc], f32, tag="e_ps",
                               name="e_ps")
            dx_blk = dx_blks[t // DXB]
            w2_use = w2_13 if lo else w2_sb
            s_use = s_sb_r if lo else s_sb
            for c in range(NCHAIN):
                cs = slice(c * Bc, (c + 1) * Bc)
                h_ps = psum_h.tile([MLP_W, Bc], f32, tag="h_ps", name="h_ps")
                nc.tensor.matmul(
                    h_ps[:], w1b_sb[:], z_st[c][0 : HID + 1, t % NSLOT, :],
                    start=True, stop=True
                )
                relu_op(kn["relu_l"][c], h_tile[:, c, :], h_ps[:])
                for j in range(NBANK):
                    nc.tensor.matmul(f_ps[j][:, c, :], w2_use[:, j, :],
                                     h_tile[:, c, :], start=True, stop=True)
                    f_sb = fpool.tile([128, Bc], f16 if lo else f32,
                                      tag=f"f_{c}_{j}", name=f"f_sb{c}_{j}")
                    nc.scalar.activation(
                        f_sb[:], f_ps[j][:, c, :], AF.Tanh,
                        bias=b2_sb[:, j : j + 1]
                    )
                    g_sb = gpool.tile([128, Bc], f32r if lo else f32,
                                      tag=f"g_{c}_{j}", name=f"g_sb{c}_{j}")
                    eng(kn["gmul_l"][c][j]).tensor_mul(
                        g_sb[:], f_sb[:], dx_blk[:, t % DXB, cs]
                    )
                    nc.tensor.matmul(e_ps[:, c, :], s_use[:, j, :], g_sb[:],
                                     start=j == 0, stop=j == NBANK - 1)
                eng(kn["add_e"][c]).tensor_add(
                    z_st[c][:, (t + 1) % NSLOT, :], e_ps[:, c, :],
                    z_st[c][:, t % NSLOT, :]
                )
            drain_out(t)

        def dma_dx(blk):
            if blk >= NDXBLK:
                return
            dx_sb = dxpool.tile([128, DXB, B], f32, tag="dx", name="dx_sb")
            nc.sync.dma_start(
                dx_sb[:],
                dxt[blk][None].to_broadcast([128 // IN_CH, IN_CH, DXB, B]),
            )
            dx_blks[blk] = dx_sb
            if blk - DXBLK_PREFETCH - 1 in dx_blks:
                del dx_blks[blk - DXBLK_PREFETCH - 1]

        def scan_body():
            init_chains()
            dx_blks.clear()
            for blk in range(DXBLK_PREFETCH):
                dma_dx(blk)
            for t in range(NSTEP):
                lo = t >= kn["t0"]
                if t % DXB == 0:
                    dma_dx(t // DXB + DXBLK_PREFETCH)
                if lo and kn["lsplit"]:
                    step_split(t, lo)
                else:
                    step_fused(t, lo)

        if repeat == 1:
            scan_body()
        else:
            # hardware loop: trip count is a runtime scalar, so timing
            # amplification costs no extra instructions
            with tc.For_i(0, repeat):
                scan_body()

    nc.compile()
    return nc


def _prep_host(u0, coeffs, W1, b1, W2, b2, Wi, bi, Wr, br):
    f32 = np.float32

    u0t_full = np.empty((IN_CH, B_FULL), f32)
    u0t_full[: IN_CH - 1] = u0.T
    u0t_full[IN_CH - 1] = 1.0

    dX = (coeffs[:, 1:] - coeffs[:, :-1]).astype(f32)  # [B_FULL, NSTEP, IN_CH]
    dxt_step = dX.transpose(1, 2, 0)  # [NSTEP, 8, B_FULL]
    dxt_pad = np.zeros((NDXBLK * DXB, IN_CH, B_FULL), f32)
    dxt_pad[:NSTEP] = dxt_step
    # [NDXBLK, 8, DXB, B_FULL]: one DMA block covers DXB scan steps
    dxt_small = np.ascontiguousarray(
        dxt_pad.reshape(NDXBLK, DXB, IN_CH, B_FULL).transpose(0, 2, 1, 3)
    )

    # z columns: 0..63 state, 64 const-one, 65 readout
    wi_mat = np.zeros((IN_CH, HID1), f32)
    wi_mat[: IN_CH - 1, :HID] = Wi
    wi_mat[IN_CH - 1, :HID] = bi
    wi_mat[IN_CH - 1, ROW_ONE] = 1.0
    wi_mat[: IN_CH - 1, ROW_OUT] = (Wi @ Wr)[:, 0]
    wi_mat[IN_CH - 1, ROW_OUT] = float(bi @ Wr[:, 0] + br[0])

    # mm1 stationary: [W1; b1] against moving z[0:65] (row 64 == 1.0)
    w1b = np.empty((HID + 1, MLP_W), f32)
    w1b[:HID] = W1
    w1b[HID] = b1

    w2_banks = np.ascontiguousarray(W2.reshape(MLP_W, NBANK, 128))
    b2_banks = np.ascontiguousarray(b2.reshape(NBANK, 128).T)

    p = np.arange(128)
    s_full = np.zeros((128, NBANK, HID1), f32)
    for j in range(NBANK):
        s_full[p, j, 16 * j + p // IN_CH] = 1.0
        s_full[p, j, ROW_OUT] = Wr[16 * j + p // IN_CH, 0]

    return {
        "u0t": u0t_full,
        "dxt": dxt_small,
        "w1b": w1b,
        "w2": w2_banks.astype(f32),
        "b2": b2_banks.astype(f32),
        "wi": wi_mat,
        "smat": s_full,
    }


def _make_in_maps(full):
    in_maps = []
    for c in range(N_CORES):
        sl = slice(c * B, (c + 1) * B)
        in_maps.append(
            {
                "u0t": np.ascontiguousarray(full["u0t"][:, sl]),
                "dxt": np.ascontiguousarray(full["dxt"][:, :, :, sl]),
                "w1b": full["w1b"],
                "w2": full["w2"],
                "b2": full["b2"],
                "wi": full["wi"],
                "smat": full["smat"],
            }
        )
    return in_maps


def kernel(u0, coeffs, W1, b1, W2, b2, Wi, bi, Wr, br, repeat=None, knobs=None):
    from concourse.bass_utils import run_bass_kernel_spmd

    full = _prep_host(
        np.asarray(u0, np.float32), np.asarray(coeffs, np.float32),
        np.asarray(W1, np.float32), np.asarray(b1, np.float32),
        np.asarray(W2, np.float32), np.asarray(b2, np.float32),
        np.asarray(Wi, np.float32), np.asarray(bi, np.float32),
        np.asarray(Wr, np.float32).reshape(HID, OUT),
        np.asarray(br, np.float32).reshape(OUT),
    )
    in_maps = _make_in_maps(full)

    rep = REPEAT if repeat is None else repeat
    key = ("nc", rep, repr(sorted(knobs.items())) if knobs else None)
    if key not in _CACHE:
        _CACHE[key] = _build_bass(rep, knobs)
    nc = _CACHE[key]

    res = run_bass_kernel_spmd(nc, in_maps, core_ids=list(range(N_CORES)))
    outs = res.results

    out_full = np.empty((B_FULL, T, OUT), np.float32)
    for c in range(N_CORES):
        out_full[c * B : (c + 1) * B, :, 0] = outs[c]["outp"].T
    return out_full


# revision 23
# speedup vs baseline: 1.3707x; 1.2131x over previous
"""Neural CDE forward pass on 8 Trainium2 NeuronCores (Bass/Tile).

Math (per batch element b):
    z0 = u0 @ Wi + bi                                   [64]
    for t in 0..164:
        h  = relu(z @ W1 + b1)                          [128]
        f  = tanh(h @ W2 + b2)                          [512] -> [64, 8]
        z += einsum('hi,i->h', f, dx_t)                 dx_t = coeffs[t+1]-coeffs[t]
    out_t = z_t @ Wr + br  for every t (166 values)

Numerics (hardware-measured, see git history of this docstring):
  - The scan is chaotic: errors amplify ~1.05x/step (~3000x over 165 steps).
  - fp32 matmul: exact-grade but 4 cycles/row; float32r: operands rounded
    to ~12 mantissa bits (1.4e-4 rel/step) at 1 cycle/row.
  - Hybrid phase split: steps t < T0=59 run fp32 mm2/reduce, t >= T0 run
    f32r.  Final error ~1e-2 rel (gate 2e-2).  T0=59 is the measured knee;
    mm1 (the z state stream) stays fp32 in both phases.

Kernel design (per core, batch shard B=512 in NCHAIN=2 chains of Bc=256):
  - State z [66, Bc] fp32: rows 0..63 state, row 64 == 1.0 (carries b1
    into mm1: stationary w1b[65,128] = [W1; b1], moving z[0:65]), row 65 =
    running readout out_t = z_t @ Wr + br.  The 1.0 row means relu needs
    NO fused bias, so it can run on any engine.  The reduce matrices S_j
    [128, 66] get a zero column at 64 (keeps the 1.0 row fixed) and the
    Wr column at 65 (readout rides the reduce for free).
  - States are slotted: state s lives at z_st[c][:, s % NSLOT, :].  Output
    row 65 drains with ONE DMA per OUTB=4 states per chain, and dx loads
    arrive DXB=4 steps per DMA (HWDGE fixed cost ~625ns/DMA; batching
    keeps the SP queue + HWDGE device off the critical path).
  - fp32 phase (t < T0): PE-bound; both chains fused into wide [*, 512]
    ops (one relu half per chain, one mm2/tanh/gmul/reduce per bank) to
    minimize PE instruction count.
  - f32r phase (t >= T0): latency-bound on the z->h->f->g->e->z cycle.
    Emission switches to per-chain ops (knob lsplit): each chain becomes
    an independent recurrence pipeline (mm2/tanh/gmul/reduce per chain
    per bank at [*, 256]); the two pipelines interleave on the engines,
    hiding each other's serial latency.  PSUM tiles keep the fused
    [.., NCHAIN, Bc] shape with per-chain half-views so both phases share
    the same PSUM budget (14KB of 16KB).
  - Engine knobs spread elementwise work: relu/gmul/z-add per chain on
    vector vs gpsimd vs scalar (tuned via the cost-model TimelineSim,
    verified on hardware).
"""

import numpy as np

IN_CH = 8
HID = 64
MLP_W = 128
OUT = 1
B_FULL, T = 4096, 166
NSTEP = T - 1
N_CORES = 8
B = B_FULL // N_CORES  # 512
NBANK = 4
HID1 = HID + 2  # 64 state rows + const-one row (64) + readout row (65)
ROW_ONE = HID
ROW_OUT = HID + 1

T0 = 59  # steps < T0 run fp32 matmuls; steps >= T0 run f32r
NCHAIN = 2
Bc = B // NCHAIN  # 256
DXB = 4  # scan steps per dx DMA block (batched to amortize HWDGE overhead)
NDXBLK = (NSTEP + DXB - 1) // DXB  # 42 (last block zero-padded)
DXBLK_PREFETCH = 2
NSLOT = 8  # z state slots per chain (state s lives at slot s % NSLOT)
OUTB = 4  # states per output DMA (4 divides NSLOT so slot runs stay contiguous)
REPEAT = 1

# engine assignment knobs ("vector" | "gpsimd" | "scalar" where noted)
KNOBS = dict(
    t0=T0,
    lsplit=True,                      # per-chain emission in the f32r phase
    relu_e="scalar",                  # fp32-phase relu engine
    # PSUM readers (relu, z-add) are restricted to vector/scalar — GPSIMD
    # has no PSUM port.  gmul reads SBUF only, so it may use gpsimd.
    relu_l=("scalar", "vector"),      # f32r-phase relu engine per chain
    gmul_e=("vector", "gpsimd", "vector", "gpsimd"),  # fp32 phase, per bank
    gmul_l=(("vector", "gpsimd", "vector", "gpsimd"),
            ("gpsimd", "vector", "gpsimd", "vector")),  # f32r, [chain][bank]
    add_e=("vector", "vector"),       # per chain, both phases
    # fp32-phase reduce split: e = S.T g_hi + S.T g_lo with g_hi=round11(g)
    # (f32r store rounding), g_lo = g - g_hi.  Replaces 4 fp32 reduce
    # matmuls (4 cyc/row) with 8 f32r ones (1 cyc/row); error ~2^-24.
    redsplit=False,
    esplit=True,                      # per-chain emission in the fp32 phase
    mm2fuse=False,                     # fuse mm2 across chains in split steps
    ghi_e=("scalar", "vector", "scalar", "vector"),   # g_hi copy engine/bank
    glo_e=("vector", "gpsimd", "vector", "gpsimd"),   # g_lo sub engine/bank
)

_CACHE = {}


def _build_bass(repeat=1, knobs=None):
    from contextlib import ExitStack

    import concourse.tile as tile
    from concourse import bacc, mybir

    kn = dict(KNOBS)
    if knobs:
        kn.update(knobs)

    f32 = mybir.dt.float32
    f32r = mybir.dt.float32r
    f16 = mybir.dt.float16
    AF = mybir.ActivationFunctionType

    nc = bacc.Bacc("TRN2", target_bir_lowering=False, debug=False)

    u0t = nc.dram_tensor("u0t", [IN_CH, B], f32, kind="ExternalInput")
    dxt = nc.dram_tensor("dxt", [NDXBLK, IN_CH, DXB, B], f32,
                         kind="ExternalInput")
    w1b = nc.dram_tensor("w1b", [HID + 1, MLP_W], f32, kind="ExternalInput")
    w2 = nc.dram_tensor("w2", [MLP_W, NBANK, 128], f32, kind="ExternalInput")
    b2 = nc.dram_tensor("b2", [128, NBANK], f32, kind="ExternalInput")
    wi = nc.dram_tensor("wi", [IN_CH, HID1], f32, kind="ExternalInput")
    smat = nc.dram_tensor("smat", [128, NBANK, HID1], f32,
                          kind="ExternalInput")
    outp = nc.dram_tensor("outp", [T, B], f32, kind="ExternalOutput")

    def eng(name):
        return getattr(nc, name)

    def relu_op(engine, dst, src):
        if engine == "scalar":
            nc.scalar.activation(dst, src, AF.Relu)
        else:
            eng(engine).tensor_relu(dst, src)

    with tile.TileContext(nc) as tc, ExitStack() as ctx:
        const = ctx.enter_context(tc.tile_pool(name="const", bufs=1))
        hpool = ctx.enter_context(tc.tile_pool(name="hpool", bufs=2))
        fpool = ctx.enter_context(tc.tile_pool(name="fpool", bufs=2))
        gpool = ctx.enter_context(tc.tile_pool(name="gpool", bufs=3))
        dxpool = ctx.enter_context(tc.tile_pool(name="dxpool", bufs=3))
        psum_h = ctx.enter_context(tc.tile_pool(name="psum_h", bufs=2, space="PSUM"))
        psum_f = ctx.enter_context(tc.tile_pool(name="psum_f", bufs=2, space="PSUM"))
        psum_e = ctx.enter_context(tc.tile_pool(name="psum_e", bufs=2, space="PSUM"))

        w1b_sb = const.tile([HID + 1, MLP_W], f32)
        nc.sync.dma_start(w1b_sb[:], w1b[:])
        w2_sb = const.tile([MLP_W, NBANK, 128], f32)
        nc.sync.dma_start(w2_sb[:], w2[:])
        b2_sb = const.tile([128, NBANK], f32)
        nc.sync.dma_start(b2_sb[:], b2[:])
        wi_sb = const.tile([IN_CH, HID1], f32)
        nc.sync.dma_start(wi_sb[:], wi[:])
        s_sb = const.tile([128, NBANK, HID1], f32)
        nc.sync.dma_start(s_sb[:], smat[:])
        s_sb_r = const.tile([128, NBANK, HID1], f32r, name="s_sb_r")
        nc.vector.tensor_copy(s_sb_r[:], s_sb[:])
        u0t_sb = const.tile([IN_CH, B], f32)
        nc.sync.dma_start(u0t_sb[:], u0t[:])

        # f32r alias of W2 for the late phase (PE rounds internally; the
        # DVE copy applies the same rounding, value-identical)
        w2_13 = const.tile([MLP_W, NBANK, 128], f32r, name="w2_13")
        nc.vector.tensor_copy(w2_13[:], w2_sb[:])

        # Slotted state: state s lives at z_st[c][:, s % NSLOT, :]
        z_st = [
            const.tile([HID1, NSLOT, Bc], f32, name=f"z_st{c}")
            for c in range(NCHAIN)
        ]
        dx_blks = {}
        g_banks = [None] * NBANK

        def init_chains():
            z0_ps = psum_e.tile([HID1, NCHAIN, Bc], f32, tag="e_ps",
                                name="z0_ps")
            for c in range(NCHAIN):
                cs = slice(c * Bc, (c + 1) * Bc)
                nc.tensor.matmul(
                    z0_ps[:, c, :], wi_sb[:], u0t_sb[:, cs],
                    start=True, stop=True
                )
                nc.vector.tensor_copy(z_st[c][:, 0, :], z0_ps[:, c, :])

        def drain_out(t):
            """one DMA per chain per OUTB states once state t+1 closes a
            group (or is the last state)"""
            s_hi = t + 1
            if s_hi % OUTB == OUTB - 1 or s_hi == NSTEP:
                s_lo = (s_hi // OUTB) * OUTB
                sl = s_lo % NSLOT
                n = s_hi - s_lo + 1
                for c in range(NCHAIN):
                    cs = slice(c * Bc, (c + 1) * Bc)
                    nc.sync.dma_start(
                        outp[s_lo : s_hi + 1, cs],
                        z_st[c][ROW_OUT : ROW_OUT + 1, sl : sl + n, :],
                    )

        def step_fused(t, lo):
            """both chains fused into wide ops (fp32 phase: PE-bound)"""
            h_tile = hpool.tile([MLP_W, NCHAIN, Bc], f32r if lo else f32,
                                tag="h_r" if lo else "h_f", name="h_tile")
            for c in range(NCHAIN):
                h_ps = psum_h.tile([MLP_W, Bc], f32, tag="h_ps", name="h_ps")
                nc.tensor.matmul(
                    h_ps[:], w1b_sb[:], z_st[c][0 : HID + 1, t % NSLOT, :],
                    start=True, stop=True
                )
                relu_op(kn["relu_e"], h_tile[:, c, :], h_ps[:])
            dx_sb = dx_blks[t // DXB][:, t % DXB, :]
            w2_use = w2_13 if lo else w2_sb
            for j in range(NBANK):
                f_ps = psum_f.tile([128, NCHAIN, Bc], f32, tag=f"f_ps{j}",
                                   bufs=1, name=f"f_ps{j}")
                nc.tensor.matmul(f_ps[:], w2_use[:, j, :], h_tile[:],
                                 start=True, stop=True)
                f_sb = fpool.tile([128, NCHAIN, Bc], f16 if lo else f32,
                                  tag=f"f_r{j}" if lo else f"f_f{j}",
                                  name=f"f_sb{j}")
                nc.scalar.activation(
                    f_sb[:], f_ps[:], AF.Tanh, bias=b2_sb[:, j : j + 1]
                )
                g_sb = gpool.tile([128, NCHAIN, Bc], f32r if lo else f32,
                                  tag=f"g_r{j}" if lo else f"g_f{j}",
                                  name=f"g_sb{j}")
                eng(kn["gmul_e"][j]).tensor_mul(g_sb[:], f_sb[:], dx_sb[:])
                g_banks[j] = g_sb
            e_ps = psum_e.tile([HID1, NCHAIN, Bc], f32, tag="e_ps",
                               name="e_ps")
            if not lo and kn["redsplit"]:
                # hi/lo split: both reduce operands f32r (1 cyc/row) with
                # fp32-grade accuracy.  g_hi = round11(g) via f32r store
                # rounding (all engines round identically, probe-verified);
                # g_lo = g - g_hi is exact (Sterbenz) and rounds at store
                # to ~2^-24 of g.  Emission order: all g_hi ops first so
                # engine FIFOs serve the spine before the trailing lo path.
                g_his, g_los = [], []
                for j in range(NBANK):
                    g_hi = gpool.tile([128, NCHAIN, Bc], f32r,
                                      tag=f"ghi{j}", name=f"g_hi{j}")
                    ge = kn["ghi_e"][j]
                    if ge == "scalar":
                        nc.scalar.copy(g_hi[:], g_banks[j][:])
                    else:
                        eng(ge).tensor_copy(g_hi[:], g_banks[j][:])
                    g_his.append(g_hi)
                    nc.tensor.matmul(e_ps[:], s_sb_r[:, j, :], g_hi[:],
                                     start=j == 0, stop=False)
                for j in range(NBANK):
                    g_lo = gpool.tile([128, NCHAIN, Bc], f32r,
                                      tag=f"glo{j}", name=f"g_lo{j}")
                    eng(kn["glo_e"][j]).tensor_sub(
                        g_lo[:], g_banks[j][:], g_his[j][:]
                    )
                    g_los.append(g_lo)
                for j in range(NBANK):
                    nc.tensor.matmul(e_ps[:], s_sb_r[:, j, :], g_los[j][:],
                                     start=False, stop=j == NBANK - 1)
            else:
                s_use = s_sb_r if lo else s_sb
                for j in range(NBANK):
                    nc.tensor.matmul(e_ps[:], s_use[:, j, :], g_banks[j][:],
                                     start=j == 0, stop=j == NBANK - 1)
            for c in range(NCHAIN):
                eng(kn["add_e"][c]).tensor_add(
                    z_st[c][:, (t + 1) % NSLOT, :], e_ps[:, c, :],
                    z_st[c][:, t % NSLOT, :]
                )
            drain_out(t)

        def step_split(t, lo):
            """per-chain emission (f32r phase: latency-bound; the chains
            form two independent pipelines).  PSUM tiles keep the fused
            shape; each chain uses its half-view."""
            h_tile = hpool.tile([MLP_W, NCHAIN, Bc], f32r if lo else f32,
                                tag="h_r" if lo else "h_f", name="h_tile")
            f_ps = [
                psum_f.tile([128, NCHAIN, Bc], f32, tag=f"f_ps{j}",
                            bufs=1, name=f"f_ps{j}")
                for j in range(NBANK)
            ]
            e_ps = psum_e.tile([HID1, NCHAIN, Bc], f32, tag="e_ps",
                               name="e_ps")
            dx_blk = dx_blks[t // DXB]
            w2_use = w2_13 if lo else w2_sb
            s_use = s_sb_r if lo else s_sb
            rsplit = not lo and kn["redsplit"]
            mm2fuse = kn["mm2fuse"]
            if mm2fuse:
                # one wide mm2 per bank (both chains) after both relus;
                # everything downstream stays per-chain
                for c in range(NCHAIN):
                    h_ps = psum_h.tile([MLP_W, Bc], f32, tag="h_ps",
                                       name="h_ps")
                    nc.tensor.matmul(
                        h_ps[:], w1b_sb[:],
                        z_st[c][0 : HID + 1, t % NSLOT, :],
                        start=True, stop=True
                    )
                    relu_op(kn["relu_l"][c], h_tile[:, c, :], h_ps[:])
                for j in range(NBANK):
                    nc.tensor.matmul(f_ps[j][:], w2_use[:, j, :], h_tile[:],
                                     start=True, stop=True)
            for c in range(NCHAIN):
                cs = slice(c * Bc, (c + 1) * Bc)
                if not mm2fuse:
                    h_ps = psum_h.tile([MLP_W, Bc], f32, tag="h_ps",
                                       name="h_ps")
                    nc.tensor.matmul(
                        h_ps[:], w1b_sb[:],
                        z_st[c][0 : HID + 1, t % NSLOT, :],
                        start=True, stop=True
                    )
                    relu_op(kn["relu_l"][c], h_tile[:, c, :], h_ps[:])
                g_los = []
                for j in range(NBANK):
                    if not mm2fuse:
                        nc.tensor.matmul(f_ps[j][:, c, :], w2_use[:, j, :],
                                         h_tile[:, c, :], start=True,
                                         stop=True)
                    f_sb = fpool.tile([128, Bc], f16 if lo else f32,
                                      tag=f"f_{c}_{j}", name=f"f_sb{c}_{j}")
                    nc.scalar.activation(
                        f_sb[:], f_ps[j][:, c, :], AF.Tanh,
                        bias=b2_sb[:, j : j + 1]
                    )
                    g_sb = gpool.tile([128, Bc], f32r if lo or rsplit else f32,
                                      tag=f"g_{c}_{j}", name=f"g_sb{c}_{j}")
                    eng(kn["gmul_l"][c][j]).tensor_mul(
                        g_sb[:], f_sb[:], dx_blk[:, t % DXB, cs]
                    )
                    if rsplit:
                        # g_sb is g_hi = round11(f*dx); recompute the
                        # product exactly and subtract for the lo term
                        g2 = gpool.tile([128, Bc], f32, tag=f"g2_{c}_{j}",
                                        name=f"g2_{c}_{j}")
                        eng(kn["ghi_e"][j] if kn["ghi_e"][j] != "scalar"
                            else "vector").tensor_mul(
                            g2[:], f_sb[:], dx_blk[:, t % DXB, cs]
                        )
                        g_lo = gpool.tile([128, Bc], f32r, tag=f"glo_{c}_{j}",
                                          name=f"glo_{c}_{j}")
                        eng(kn["glo_e"][j]).tensor_sub(g_lo[:], g2[:], g_sb[:])
                        g_los.append(g_lo)
                        nc.tensor.matmul(e_ps[:, c, :], s_sb_r[:, j, :],
                                         g_sb[:], start=j == 0, stop=False)
                    else:
                        nc.tensor.matmul(e_ps[:, c, :], s_use[:, j, :],
                                         g_sb[:], start=j == 0,
                                         stop=j == NBANK - 1)
                if rsplit:
                    for j in range(NBANK):
                        nc.tensor.matmul(e_ps[:, c, :], s_sb_r[:, j, :],
                                         g_los[j][:], start=False,
                                         stop=j == NBANK - 1)
                eng(kn["add_e"][c]).tensor_add(
                    z_st[c][:, (t + 1) % NSLOT, :], e_ps[:, c, :],
                    z_st[c][:, t % NSLOT, :]
                )
            drain_out(t)

        def dma_dx(blk):
            if blk >= NDXBLK:
                return
            dx_sb = dxpool.tile([128, DXB, B], f32, tag="dx", name="dx_sb")
            nc.sync.dma_start(
                dx_sb[:],
                dxt[blk][None].to_broadcast([128 // IN_CH, IN_CH, DXB, B]),
            )
            dx_blks[blk] = dx_sb
            if blk - DXBLK_PREFETCH - 1 in dx_blks:
                del dx_blks[blk - DXBLK_PREFETCH - 1]

        def scan_body():
            init_chains()
            dx_blks.clear()
            for blk in range(DXBLK_PREFETCH):
                dma_dx(blk)
            for t in range(NSTEP):
                lo = t >= kn["t0"]
                if t % DXB == 0:
                    dma_dx(t // DXB + DXBLK_PREFETCH)
                if kn["lsplit"] if lo else kn["esplit"]:
                    step_split(t, lo)
                else:
                    step_fused(t, lo)

        if repeat == 1:
            scan_body()
        else:
            # hardware loop: trip count is a runtime scalar, so timing
            # amplification costs no extra instructions
            with tc.For_i(0, repeat):
                scan_body()

    nc.compile()
    return nc


def _prep_host(u0, coeffs, W1, b1, W2, b2, Wi, bi, Wr, br):
    f32 = np.float32

    u0t_full = np.empty((IN_CH, B_FULL), f32)
    u0t_full[: IN_CH - 1] = u0.T
    u0t_full[IN_CH - 1] = 1.0

    dX = (coeffs[:, 1:] - coeffs[:, :-1]).astype(f32)  # [B_FULL, NSTEP, IN_CH]
    dxt_step = dX.transpose(1, 2, 0)  # [NSTEP, 8, B_FULL]
    dxt_pad = np.zeros((NDXBLK * DXB, IN_CH, B_FULL), f32)
    dxt_pad[:NSTEP] = dxt_step
    # [NDXBLK, 8, DXB, B_FULL]: one DMA block covers DXB scan steps
    dxt_small = np.ascontiguousarray(
        dxt_pad.reshape(NDXBLK, DXB, IN_CH, B_FULL).transpose(0, 2, 1, 3)
    )

    # z columns: 0..63 state, 64 const-one, 65 readout
    wi_mat = np.zeros((IN_CH, HID1), f32)
    wi_mat[: IN_CH - 1, :HID] = Wi
    wi_mat[IN_CH - 1, :HID] = bi
    wi_mat[IN_CH - 1, ROW_ONE] = 1.0
    wi_mat[: IN_CH - 1, ROW_OUT] = (Wi @ Wr)[:, 0]
    wi_mat[IN_CH - 1, ROW_OUT] = float(bi @ Wr[:, 0] + br[0])

    # mm1 stationary: [W1; b1] against moving z[0:65] (row 64 == 1.0)
    w1b = np.empty((HID + 1, MLP_W), f32)
    w1b[:HID] = W1
    w1b[HID] = b1

    w2_banks = np.ascontiguousarray(W2.reshape(MLP_W, NBANK, 128))
    b2_banks = np.ascontiguousarray(b2.reshape(NBANK, 128).T)

    p = np.arange(128)
    s_full = np.zeros((128, NBANK, HID1), f32)
    for j in range(NBANK):
        s_full[p, j, 16 * j + p // IN_CH] = 1.0
        s_full[p, j, ROW_OUT] = Wr[16 * j + p // IN_CH, 0]

    return {
        "u0t": u0t_full,
        "dxt": dxt_small,
        "w1b": w1b,
        "w2": w2_banks.astype(f32),
        "b2": b2_banks.astype(f32),
        "wi": wi_mat,
        "smat": s_full,
    }


def _make_in_maps(full):
    in_maps = []
    for c in range(N_CORES):
        sl = slice(c * B, (c + 1) * B)
        in_maps.append(
            {
                "u0t": np.ascontiguousarray(full["u0t"][:, sl]),
                "dxt": np.ascontiguousarray(full["dxt"][:, :, :, sl]),
                "w1b": full["w1b"],
                "w2": full["w2"],
                "b2": full["b2"],
                "wi": full["wi"],
                "smat": full["smat"],
            }
        )
    return in_maps


def kernel(u0, coeffs, W1, b1, W2, b2, Wi, bi, Wr, br, repeat=None, knobs=None):
    from concourse.bass_utils import run_bass_kernel_spmd

    full = _prep_host(
        np.asarray(u0, np.float32), np.asarray(coeffs, np.float32),
        np.asarray(W1, np.float32), np.asarray(b1, np.float32),
        np.asarray(W2, np.float32), np.asarray(b2, np.float32),
        np.asarray(Wi, np.float32), np.asarray(bi, np.float32),
        np.asarray(Wr, np.float32).reshape(HID, OUT),
        np.asarray(br, np.float32).reshape(OUT),
    )
    in_maps = _make_in_maps(full)

    rep = REPEAT if repeat is None else repeat
    key = ("nc", rep, repr(sorted(knobs.items())) if knobs else None)
    if key not in _CACHE:
        _CACHE[key] = _build_bass(rep, knobs)
    nc = _CACHE[key]

    res = run_bass_kernel_spmd(nc, in_maps, core_ids=list(range(N_CORES)))
    outs = res.results

    out_full = np.empty((B_FULL, T, OUT), np.float32)
    for c in range(N_CORES):
        out_full[c * B : (c + 1) * B, :, 0] = outs[c]["outp"].T
    return out_full
